# revision 23
# baseline (speedup 1.0000x reference)
"""GQA causal attention (B=1, T=4096, D=1024, HQ=16, HKV=4, HD=64) on 8 trn2
NeuronCores via Bass/Tile.

Sharding: block-cyclic sequence-parallel. The 4096 query tokens are split into
64 blocks of 64; core i owns blocks {i, 8+i, ..., 56+i} (512 q tokens). Every
core runs the SAME program (SPMD requirement): for its j-th block it processes
k-tiles [0, 4*(j+1)) — a core-independent conservative causal extent — and a
host-supplied per-core boundary mask zeroes the non-causal tail, so per-core
work is uniform AND balanced (each core ~1/8 of the causal area).

Layout strategy (avoids all on-device transposes):
  - host passes x^T; scores are computed as S^T[k, q] = (K^T)^T-tiles @ Q^T
    with k on partitions, so the softmax denominator is obtained by appending
    a ones-column to the V stationary ([V|1]) and the exp is a pure
    elementwise ACT pass PSUM->SBUF.
  - normalization is deferred: ctx^T = (sum_k e^s V) is divided by the
    rowsum (row 64 of the [V|1] matmul output) after the k-loop, via a
    reciprocal + K=1 broadcast-matmul.
  - Wq/Wo columns/rows are host-permuted so two heads stack into 128
    partitions everywhere (even-kv heads on partitions 0-63, odd-kv on
    64-127), which also lets score matmuls (contraction dim = head_dim = 64)
    run pairwise-packed in the PE array via tile_position row groups.

dtypes: projections and the output matmul run in float32r (full-rate fp32
variant, moving dim >= 256); score/ctx matmuls run in bf16 (N=64 would put
f32r in its slow mode); softmax accumulation is fp32 in PSUM.
"""

import os
import sys

sys.path.insert(0, "/opt/trn_rl_repo")

import numpy as np
import ml_dtypes

import concourse.bass as bass
import concourse.bacc as bacc
import concourse.mybir as mybir
import concourse.tile as tile
from concourse.bass_utils import run_bass_kernel_spmd

# ---------------------------------------------------------------- constants
B, T, D = 1, 4096, 1024
HQ, HKV, HD = 16, 4, 64
G = HQ // HKV          # 4 q heads per kv head
NC = 8                 # cores
QB = 64                # q block size
NBLK = T // QB         # 64 blocks total
BPC = NBLK // NC       # 8 blocks per core
LQ = QB * BPC          # 512 local q tokens per core
DT = D // 128          # 8 contraction tiles over D
NKT = T // 128         # 32 k-tiles
F32 = mybir.dt.float32
F32R = mybir.dt.float32r
BF16 = mybir.dt.bfloat16
BF16NP = ml_dtypes.bfloat16

# head pairing: pair tile m holds (LO[m] on partitions 0-63, HI[m] on 64-127).
# LO = heads of even kv-heads, HI = heads of odd kv-heads, so score matmuls of
# a lo head (stationary K^T at partitions 0-63) can be row-group-packed with a
# hi head (partitions 64-127).
LO = [0, 1, 2, 3, 8, 9, 10, 11]
HI = [4, 5, 6, 7, 12, 13, 14, 15]


def _local_cols(i):
    """Global token indices owned by core i, in local order."""
    return np.concatenate(
        [QB * (NC * j + i) + np.arange(QB) for j in range(BPC)]
    )


def _band_mask(i):
    """[4, 128, 64] multiplicative causal mask for the last k-quartet of any
    block: valid iff 128*kt2 + p <= 64*i + f."""
    kt2 = np.arange(4)[:, None, None]
    p = np.arange(128)[None, :, None]
    f = np.arange(64)[None, None, :]
    return (128 * kt2 + p <= 64 * i + f).astype(BF16NP)


def _r(ap):
    return ap.bitcast(F32R)


# ---------------------------------------------------------------- program
def build_nc():
    nc = bacc.Bacc(None)
    xo_d = nc.declare_dram_parameter("xT_own", [D, LQ], F32R, isOutput=False)
    xf_d = nc.declare_dram_parameter("xT_full", [D, T], BF16, isOutput=False)
    wq_d = nc.declare_dram_parameter("Wq_perm", [D, HQ * HD], F32R, isOutput=False)
    wk_d = nc.declare_dram_parameter("Wk_n", [D, HKV * HD], BF16, isOutput=False)
    wv_d = nc.declare_dram_parameter("Wv_n", [D, HKV * HD], BF16, isOutput=False)
    wo_d = nc.declare_dram_parameter("Wo_perm", [HQ * HD, D], F32R, isOutput=False)
    bm_d = nc.declare_dram_parameter("bmask", [4, 128, QB], BF16, isOutput=False)
    out_d = nc.declare_dram_parameter("out_loc", [LQ, D], F32, isOutput=True)

    with tile.TileContext(nc) as tc:
        _emit(nc, tc, xo_d, xf_d, wq_d, wk_d, wv_d, wo_d, bm_d, out_d)
    nc.finalize()
    return nc


def _emit(nc, tc, xo_d, xf_d, wq_d, wk_d, wv_d, wo_d, bm_d, out_d):
    from contextlib import ExitStack

    es = ExitStack()
    with es:
        sb = es.enter_context(tc.tile_pool(name="sb", bufs=2))
        sb3 = es.enter_context(tc.tile_pool(name="sb3", bufs=3))
        res = es.enter_context(tc.tile_pool(name="res", bufs=1))
        ps2 = es.enter_context(tc.tile_pool(name="ps2", bufs=2, space="PSUM"))

        # ---------------- resident tensors
        xo = res.tile([128, DT, LQ], F32R, tag="xo")          # x^T own cols
        nc.sync.dma_start(xo[:], xo_d.rearrange("(dt p) q -> p dt q", p=128))
        wk = res.tile([128, DT, HKV * HD], BF16, tag="wk")
        nc.sync.dma_start(wk[:], wk_d.rearrange("(dt p) h -> p dt h", p=128))
        wv = res.tile([128, DT, HKV * HD], BF16, tag="wv")
        nc.sync.dma_start(wv[:], wv_d.rearrange("(dt p) h -> p dt h", p=128))
        bm = res.tile([128, 4, QB], BF16, tag="bm")          # band masks
        nc.sync.dma_start(bm[:], bm_d.rearrange("k p f -> p k f"))

        kt_sb = [res.tile([128, T], BF16, tag=f"kt{h2}", name=f"kt{h2}") for h2 in range(2)]
        v_sb = res.tile([128, NKT, HKV, HD + 1], BF16, tag="v")  # [V | 1]
        qt_sb = [res.tile([128, LQ], BF16, tag=f"qt{m}", name=f"qt{m}") for m in range(8)]
        ctx_sb = res.tile([128, 8, LQ], F32R, tag="ctx")      # normalized ctx^T
        ones_sb = res.tile([1, HD], F32R, tag="ones")
        nc.vector.memset(ones_sb[:], 1.0)
        nc.vector.memset(v_sb[:, :, :, HD : HD + 1], 1.0)

        # ---------------- P1a: Q^T projection (f32r), scaled by HD^-0.5
        # Wq lives in the "wbig" slot; Wo reuses the same slot later (the
        # phases are sequential, the pool dependency-orders the reuse).
        wqt = sb.tile([128, DT, HQ * HD], F32R, tag="wbig", name="wqt")
        nc.sync.dma_start(wqt[:], wq_d.rearrange("(dt p) h -> p dt h", p=128))
        for m in range(8):
            psq = ps2.tile([128, LQ], F32, tag="pacc", name=f"psq{m}")
            for d in range(DT):
                nc.tensor.matmul(
                    psq[:],
                    wqt[:, d, 128 * m : 128 * (m + 1)],
                    xo[:, d, :],
                    start=(d == 0),
                    stop=(d == DT - 1),
                )
            # cast to bf16 with the 1/sqrt(HD) score scale folded in
            nc.vector.tensor_scalar_mul(qt_sb[m][:], psq[:], float(HD) ** -0.5)

        # ---------------- P1b/c: K^T and V projections from bf16 x^T (full T)
        for c in range(8):  # 512-token chunks
            xf = sb.tile([128, DT, 512], BF16, tag="xf")
            nc.sync.dma_start(
                xf[:],
                xf_d.rearrange("(dt p) t -> p dt t", p=128)[:, :, 512 * c : 512 * (c + 1)],
            )
            for h2 in range(2):  # K^T: kv-pair tiles (kv0|kv1), (kv2|kv3)
                psk = ps2.tile([128, 512], F32, tag="scores", name="psk")
                for d in range(DT):
                    nc.tensor.matmul(
                        psk[:],
                        wk[:, d, 128 * h2 : 128 * (h2 + 1)],
                        xf[:, d, :],
                        start=(d == 0),
                        stop=(d == DT - 1),
                    )
                nc.vector.tensor_copy(kt_sb[h2][:, 512 * c : 512 * (c + 1)], psk[:])
            for tq in range(4):  # V natural [t, d] via x^T-stationary matmuls
                kt = 4 * c + tq
                psv = ps2.tile([128, HKV * HD], F32, tag="scores", name="psv")
                for d in range(DT):
                    nc.tensor.matmul(
                        psv[:],
                        xf[:, d, 128 * tq : 128 * (tq + 1)],
                        wv[:, d, :],
                        start=(d == 0),
                        stop=(d == DT - 1),
                    )
                nc.vector.tensor_copy(
                    v_sb[:, kt, :, 0:HD],
                    psv.rearrange("p (h e) -> p h e", h=HKV),
                )

        # ---------------- P2: attention over blocks
        for j in range(BPC):
            nkp = 2 * (j + 1)  # k-tile pairs this block
            ctx_ps = [
                ps2.tile([HD + 1, 8 * QB], F32, tag="ctx", name=f"ctxps{h2}_{j}")
                for h2 in range(2)
            ]
            for kp in range(nkp):
                for h2 in range(2):
                    s_ps = ps2.tile([128, 2, 4, 2, QB], F32, tag="scores")
                    for mq in range(4):
                        qv = qt_sb[4 * h2 + mq]
                        for kt2 in range(2):
                            kt = 2 * kp + kt2
                            ksl = slice(128 * kt, 128 * (kt + 1))
                            qsl = slice(QB * j, QB * (j + 1))
                            nc.tensor.matmul(
                                s_ps[:, 0, mq, kt2, :],
                                kt_sb[h2][0:64, ksl],
                                qv[0:64, qsl],
                                start=True, stop=True,
                                tile_position=(0, 0),
                            )
                            nc.tensor.matmul(
                                s_ps[:, 1, mq, kt2, :],
                                kt_sb[h2][64:128, ksl],
                                qv[64:128, qsl],
                                start=True, stop=True,
                                tile_position=(64, 0),
                            )
                    pt = sb3.tile([128, 2, 4, 2, QB], BF16, tag="pt")
                    nc.scalar.activation(
                        pt[:], s_ps[:], mybir.ActivationFunctionType.Exp
                    )
                    if kp >= 2 * j:  # boundary quartet: apply causal mask
                        par = kp - 2 * j
                        msk = bm[:, None, 2 * par : 2 * par + 2, :].to_broadcast(
                            (128, 4, 2, QB)
                        )
                        for hs in range(2):
                            nc.vector.tensor_mul(
                                pt[:, hs], pt[:, hs], msk
                            )
                    for kt2 in range(2):
                        kt = 2 * kp + kt2
                        for hs in range(2):
                            kv = 2 * h2 + hs
                            for mq in range(4):
                                s = 4 * hs + mq
                                # start=True only on the very first matmul into
                                # this psum tile: start marks the whole 2KB
                                # zero-region pending-zero, so each slot's first
                                # write overwrites (correct) and later writes
                                # accumulate. A second start=True would re-mark
                                # the bank and wipe other slots' partials.
                                nc.tensor.matmul(
                                    ctx_ps[h2][:, QB * s : QB * (s + 1)],
                                    v_sb[:, kt, kv, :],
                                    pt[:, hs, mq, kt2, :],
                                    start=(kp == 0 and kt2 == 0 and s == 0),
                                    stop=(kp == nkp - 1 and kt2 == 1),
                                    skip_group_check=True,
                                )
            # ---- normalize: ctx / rowsum (row HD of ctx_ps)
            rs = sb.tile([1, 2, 8 * QB], F32R, tag="rs")
            for h2 in range(2):
                nc.vector.tensor_copy(rs[0:1, h2, :], ctx_ps[h2][HD : HD + 1, :])
            with nc.allow_low_precision(
                reason="f32r rowsum reciprocal; f32r keeps 32-bit layout"
            ):
                nc.vector.reciprocal(rs[:], rs[:])
            hi_st = sb.tile([64, 8, QB], F32R, tag="hist")
            for h2 in range(2):
                bc = ps2.tile([HD, 8 * QB], F32, tag="pacc", name="bc")
                nc.tensor.matmul(
                    bc[:], ones_sb[:], rs[0:1, h2, :],
                    start=True, stop=True,
                )
                bcs = sb.tile([HD, 8 * QB], F32, tag="bcs")
                nc.vector.tensor_copy(bcs[:], bc[:])
                for hs in range(2):
                    for mq in range(4):
                        s = 4 * hs + mq
                        m = 4 * h2 + mq
                        ssl = slice(QB * s, QB * (s + 1))
                        if hs == 0:
                            nc.vector.tensor_mul(
                                ctx_sb[0:64, m, QB * j : QB * (j + 1)],
                                ctx_ps[h2][0:HD, ssl],
                                bcs[:, ssl],
                            )
                        else:
                            nc.vector.tensor_mul(
                                hi_st[:, m, :], ctx_ps[h2][0:HD, ssl], bcs[:, ssl]
                            )
            # partition-shift the odd-kv heads to partitions 64-127 (DMA)
            nc.sync.dma_start(
                ctx_sb[64:128, :, QB * j : QB * (j + 1)], hi_st[:]
            )

        # ---------------- P3: out = ctx @ Wo  (f32r)
        out_sb = res.tile([128, 4, D], F32, tag="osb")
        wot = sb.tile([128, 8, D], F32R, tag="wbig", name="wot")
        nc.sync.dma_start(wot[:], wo_d.rearrange("(m p) dcol -> p m dcol", p=128))
        for tt in range(4):
            for dc in range(2):
                pso = ps2.tile([128, 512], F32, tag="pacc", name=f"pso{tt}_{dc}")
                for m in range(8):
                    nc.tensor.matmul(
                        pso[:],
                        ctx_sb[:, m, 128 * tt : 128 * (tt + 1)],
                        wot[:, m, 512 * dc : 512 * (dc + 1)],
                        start=(m == 0),
                        stop=(m == 7),
                    )
                nc.vector.tensor_copy(
                    out_sb[:, tt, 512 * dc : 512 * (dc + 1)], pso[:]
                )
        nc.sync.dma_start(
            out_d.rearrange("(tt p) dcol -> p tt dcol", p=128), out_sb[:]
        )


def _install_ntff_hook():
    """Provide antenv.axon_hooks (absent from this image's antenv) so that
    run_bass_kernel_spmd(trace=True) can NTFF-profile via libaxon_pjrt."""
    import sys as _sys
    import types as _types

    if "antenv.axon_hooks" not in _sys.modules:
        import antenv as _antenv

        mod = _types.ModuleType("antenv.axon_hooks")
        mod._HOOK = None

        def _set(h, _m=mod):
            _m._HOOK = h

        def _get(_m=mod):
            return _m._HOOK

        mod.set_axon_ntff_profile_hook = _set
        mod.get_axon_ntff_profile_hook = _get
        _sys.modules["antenv.axon_hooks"] = mod
        _antenv.axon_hooks = mod
    mod = _sys.modules["antenv.axon_hooks"]
    if mod.get_axon_ntff_profile_hook() is None:
        import trn_agent_boot.trn_boot as _tb

        hook = _tb._ntff_profile_via_ctypes("/opt/axon/libaxon_pjrt.so")
        mod.set_axon_ntff_profile_hook(hook)
    # artifact upload needs a bucket this sandbox doesn't have
    from concourse import bass_utils as _bu

    _bu.upload_artifacts = lambda tmpdir: f"local://{tmpdir}"


# ---------------------------------------------------------------- host side
_NC_CACHE = None


def _get_nc():
    global _NC_CACHE
    if _NC_CACHE is None:
        _NC_CACHE = build_nc()
    return _NC_CACHE


def _prep_in_maps(x, Wq, Wk, Wv, Wo):
    xT = np.ascontiguousarray(x[0].T).astype(np.float32)          # [D, T]
    xT_bf = xT.astype(BF16NP)
    wq_perm = np.empty_like(Wq)
    wo_perm = np.empty_like(Wo)
    for m in range(8):
        wq_perm[:, 128 * m : 128 * m + 64] = Wq[:, 64 * LO[m] : 64 * LO[m] + 64]
        wq_perm[:, 128 * m + 64 : 128 * m + 128] = Wq[:, 64 * HI[m] : 64 * HI[m] + 64]
        wo_perm[128 * m : 128 * m + 64, :] = Wo[64 * LO[m] : 64 * LO[m] + 64, :]
        wo_perm[128 * m + 64 : 128 * m + 128, :] = Wo[64 * HI[m] : 64 * HI[m] + 64, :]
    wk_bf = Wk.astype(BF16NP)
    wv_bf = Wv.astype(BF16NP)
    maps = []
    for i in range(NC):
        cols = _local_cols(i)
        maps.append(
            {
                "xT_own": np.ascontiguousarray(xT[:, cols]),
                "xT_full": xT_bf,
                "Wq_perm": wq_perm,
                "Wk_n": wk_bf,
                "Wv_n": wv_bf,
                "Wo_perm": wo_perm,
                "bmask": _band_mask(i),
            }
        )
    return maps


def kernel(x, Wq, Wk, Wv, Wo):
    nc = _get_nc()
    maps = _prep_in_maps(
        np.asarray(x, np.float32),
        np.asarray(Wq, np.float32),
        np.asarray(Wk, np.float32),
        np.asarray(Wv, np.float32),
        np.asarray(Wo, np.float32),
    )
    trace = bool(int(os.environ.get("KERNEL_TRACE", "0")))
    if trace:
        try:
            _install_ntff_hook()
        except Exception as e:  # profiling is best-effort
            print(f"ntff hook install failed: {e}")
    r = run_bass_kernel_spmd(nc, maps, list(range(NC)), trace=trace)
    out = np.empty((B, T, D), np.float32)
    for i in range(NC):
        out[0, _local_cols(i), :] = r.results[i]["out_loc"]
    if trace:
        kernel.last_exec_time_ns = r.exec_time_ns
        kernel.last_results = r
    return out


if __name__ == "__main__":
    # quick single-core simulation check against a small numpy reference
    pass
